# revision 22
# baseline (speedup 1.0000x reference)
"""Sharded brute-force kNN (cosine-sim top-k) on 8 Trainium2 NeuronCores.

Strategy (passage-row-wise sharding, fp8 DoubleRow, passage-stationary):
  - Each core gets a 32768-passage shard (of 262144) plus the full 2048
    queries, cast to fp8e4m3 host-side and laid out K-major; the passage
    shard is additionally pre-tiled into contiguous per-DMA-group blocks
    so every input DMA streams at full HBM bandwidth (25MB + 2MB per
    core).
  - Device: S = P_shard @ Q.T as fp8 DoubleRow matmuls with the PASSAGE
    tile stationary: per 128-passage chunk, 3 k-pair weight loads each
    streamed by 4 query blocks of 512 (12 MMs of 216ns = the fp8-DR
    streaming floor). Output [128, 2048] fp8 per pchunk is cast out of
    PSUM by ACT/DVE alternately and DMA'd every ~2.6us - a uniform
    output stream (no bursts, ~1.5us tail) vs the query-stationary
    variant's 8.4MB end-of-group bursts.
  - Host: maps fp8 sims to order-preserving uint8 keys, cuts each query
    at a sampled threshold (~1.5k survivors; the exact top-k is a subset
    with tens-of-sigma margin vs the ~1-sigma fp8 matmul noise),
    rescores every survivor exactly in fp32 (blocked BLAS GEMM), and
    takes the exact top-k with jax.lax.top_k tie-breaking (lowest index
    first).
"""
import os
import time as _time

import numpy as np

import concourse.bacc as bacc
import concourse.tile as tile
from concourse import mybir
from concourse.bass_utils import run_bass_kernel_spmd

P = 128
Q = 2048              # queries (replicated on all cores)
D = 768               # embedding dim = 6 k-tiles of 128
NCORES = 8
NTOTAL = 262144       # total passages
NSH = NTOTAL // NCORES  # 32768 passages per core
CHUNK = 512           # queries per PSUM bank (moving dim)
NQG = Q // CHUNK      # 4 query groups
NPC = NSH // P        # 256 passage chunks per core
PG = 4                # passage chunks per input DMA group
KT = D // P           # 6 k-tiles

TRACE = False         # set True (e.g. from test.py) to capture an NTFF profile
LAST_PERF = None      # BassKernelResults of the last run when TRACE was set

RESCORE8 = 1024       # target survivor count per query for the host rescore

_NC_CACHE = {}


def _build_fp8():
    FP8 = mybir.dt.float8e4
    nc = bacc.Bacc("TRN2", target_bir_lowering=False)
    qt = nc.dram_tensor("qt", [D, Q], FP8, kind="ExternalInput")
    # pt arrives pre-tiled host-side as [NG, 128, 6, PG*128] so every
    # group DMA reads one contiguous block (3KB/partition runs) instead
    # of 768 separate 1KB strided strips - 8.3us -> 1.1us per group DMA
    # on the single queue.
    pt = nc.dram_tensor("pt", [NPC // PG, P, KT, PG * P], FP8, kind="ExternalInput")
    sims = nc.dram_tensor("sims", [NSH, Q], FP8, kind="ExternalOutput")

    qt_ap = qt.ap().rearrange("(s p) q -> p s q", p=P)   # [128, 6, 2048]

    DR = mybir.MatmulPerfMode.DoubleRow
    NG = NPC // PG  # input DMA groups

    with tile.TileContext(nc) as tc:
        with (
            tc.tile_pool(name="qpool", bufs=1) as qpool,
            tc.tile_pool(name="ppool", bufs=4) as ppool,
            tc.tile_pool(name="spool", bufs=4) as spool,
            tc.tile_pool(name="pspool", bufs=8, space="PSUM") as pspool,
        ):
            # PE warm-up: dummy DoubleRow matmuls on a memset scratch tile
            # so the HAM clock-gate reaches 8/8 while the first input DMAs
            # (~12us incl. semaphore-post latency) are still in flight -
            # the real matmul stream then starts at full 2.4GHz instead of
            # paying ~2us of cold-clock tax.
            warm = qpool.tile([P, 2, CHUNK], FP8, name="warm")
            nc.gpsimd.memset(warm[:], 0)
            wps = pspool.tile([P, CHUNK], mybir.dt.float32, tag="ps", name="ps")
            for _ in range(9):
                nc.tensor.matmul(
                    wps[:], warm[:, :, 0:P], warm[:],
                    start=True, stop=True, perf_mode=DR,
                )

            # Input DMAs: queries go through the SCALAR engine's DGE queue
            # in 12 k-pair/column pieces while the passage groups use the
            # SYNC queue - the two first-MM dependencies (first query
            # piece + passage group 0) issue in parallel instead of
            # serializing their ~0.65us descriptor-generation on one
            # queue, and query loading never queues behind output DMAs.
            qt_t = qpool.tile([P, KT, Q], FP8, name="qt_t")
            cur = ppool.tile([P, KT, PG * P], FP8, tag="pt", name="pt_t")
            nc.sync.dma_start(cur[:], pt.ap()[0])
            for kk in range(KT // 2):
                for j in range(NQG):
                    sl = (slice(None), slice(2 * kk, 2 * kk + 2),
                          slice(j * CHUNK, (j + 1) * CHUNK))
                    nc.scalar.dma_start(qt_t[sl], qt_ap[sl])

            for g in range(NG):
                if g + 1 < NG:
                    nxt = ppool.tile([P, KT, PG * P], FP8, tag="pt", name="pt_t")
                    nc.sync.dma_start(nxt[:], pt.ap()[g + 1])
                else:
                    nxt = None
                for pi in range(PG):
                    pglob = g * PG + pi
                    st = spool.tile([P, Q], FP8, tag="st", name="st")
                    ps = [pspool.tile([P, CHUNK], mybir.dt.float32, tag="ps", name="ps")
                          for _ in range(NQG)]
                    for kk in range(KT // 2):
                        w = cur[:, 2 * kk:2 * kk + 2, pi * P:(pi + 1) * P]
                        for j in range(NQG):
                            nc.tensor.matmul(
                                ps[j][:], w,
                                qt_t[:, 2 * kk:2 * kk + 2, j * CHUNK:(j + 1) * CHUNK],
                                start=(kk == 0), stop=(kk == KT // 2 - 1),
                                perf_mode=DR,
                            )
                    last = pglob == NPC - 1
                    for j in range(NQG):
                        dst = st[:, j * CHUNK:(j + 1) * CHUNK]
                        # alternate the PSUM->SBUF cast between ACT and DVE
                        if j % 2 == 0:
                            nc.scalar.copy(dst, ps[j][:])
                        else:
                            nc.vector.tensor_copy(dst, ps[j][:])
                        if last:
                            # final pchunk: ship each 512-query slice as its
                            # cast lands (alternating queues) so the
                            # kernel-end barrier waits on a 64KB DMA, not a
                            # 256KB one
                            eng = nc.sync if j % 2 == 0 else nc.scalar
                            eng.dma_start(
                                sims.ap()[pglob * P:(pglob + 1) * P,
                                          j * CHUNK:(j + 1) * CHUNK], dst)
                    if not last:
                        nc.sync.dma_start(
                            sims.ap()[pglob * P:(pglob + 1) * P, :], st[:])
                cur = nxt
    nc.compile()
    return nc


def _fp8_sort_keys(a):
    """Order-preserving fp8 -> uint8 map (no NaNs expected)."""
    u = a.view(np.uint8)
    flip = (u >> 7) * np.uint8(0x7F) + np.uint8(0x80)
    return u ^ flip


_PT = [0.0, 0.0]


def _t(label, t0):
    if os.environ.get("KNN_TIMING"):
        pt = _time.process_time()
        tt = _time.thread_time()
        print(f"[knn] {label}: {_time.time() - t0:.2f}s "
              f"(proc {pt - _PT[0]:.2f}s thread {tt - _PT[1]:.2f}s)", flush=True)
        _PT[0] = pt
        _PT[1] = tt
    return _time.time()


def kernel(query_embed, passage_embed, top_k):
    global LAST_PERF, _NC_CACHE
    t0 = _time.time()
    q = np.ascontiguousarray(np.asarray(query_embed, dtype=np.float32))
    p = np.asarray(passage_embed, dtype=np.float32)
    k = int(top_k)
    assert q.shape == (Q, D) and p.shape == (NTOTAL, D), (q.shape, p.shape)
    assert 1 <= k <= 128, k

    if "fp8dr" not in _NC_CACHE:
        _NC_CACHE["fp8dr"] = _build_fp8()
    nc = _NC_CACHE["fp8dr"]
    t0 = _t("build", t0)

    NP8 = mybir.dt.np(mybir.dt.float8e4)
    q8 = q.astype(NP8)
    p8u = p.astype(NP8).view(np.uint8)
    qt = np.ascontiguousarray(q8.T)
    # device pt layout: [NG, 128, 6, PG*128] with
    # pt[g, pp, s, gi*128 + c] = shard[(g*PG + gi)*128 + c, s*128 + pp]
    in_maps = []
    for c in range(NCORES):
        sh = p8u[c * NSH:(c + 1) * NSH]                 # [32768, 768]
        ptt = np.ascontiguousarray(
            sh.reshape(NPC // PG, PG * P, KT, P).transpose(0, 3, 2, 1)
        ).view(NP8)
        in_maps.append({"qt": qt, "pt": ptt})
    t0 = _t("input prep", t0)
    out = run_bass_kernel_spmd(nc, in_maps, core_ids=list(range(NCORES)), trace=TRACE)
    if TRACE:
        LAST_PERF = out
    t0 = _t("device run (incl neff compile + transfers)", t0)

    # sims arrive [NSH, Q] per core (passage-major). Work per-core to
    # avoid a 536MB host transpose.
    if os.environ.get("KNN_TIMING"):
        s0d = out.results[0]["sims"]
        a0 = np.asarray(s0d)
        print(f"[knn]   sims type={type(s0d).__name__} dtype={a0.dtype} "
              f"c_contig={a0.flags['C_CONTIGUOUS']} strides={a0.strides}",
              flush=True)
    keys_list = [_fp8_sort_keys(np.asarray(out.results[c]["sims"]))
                 for c in range(NCORES)]
    t0 = _t("keys", t0)
    # Per-query key threshold from a passage sample (first 16384 passages
    # = first half of core 0's shard), aiming for ~1.5*RESCORE8
    # survivors; statistically the true top-k is always a subset (fp8
    # matmul noise sigma ~1, fp8 key buckets ~8 wide at the boundary, vs
    # tens-of-sigma margins in the cut). Queries whose survivor count
    # lands low get an exact top-RESCORE8 fallback.
    m = RESCORE8
    S = NTOTAL // 16
    skth = (3 * m) // (2 * 16)
    th = np.partition(keys_list[0][:S], S - skth, axis=0)[S - skth]  # [Q]
    t0 = _t("  th partition", t0)
    masks = [kk >= th[None, :] for kk in keys_list]
    t0 = _t("  masks", t0)
    counts = masks[0].sum(axis=0, dtype=np.int32)
    for mk in masks[1:]:
        counts += mk.sum(axis=0, dtype=np.int32)
    t0 = _t("  counts", t0)
    bad = np.nonzero((counts < max(2 * k, 256)) | (counts > 16 * m))[0]
    if os.environ.get("KNN_TIMING"):
        print(f"[knn]   bad rows: {len(bad)}; counts min/med/max: "
              f"{counts.min()}/{int(np.median(counts))}/{counts.max()}", flush=True)
    for r in bad:  # rare (sampling tail); re-cut the query at its exact
        # m-th largest key, tie-inclusive so equal keys are all kept
        col = np.concatenate([kk[:, r] for kk in keys_list])
        th_r = np.partition(col, NTOTAL - m)[NTOTAL - m]
        for c in range(NCORES):
            masks[c][:, r] = keys_list[c][:, r] >= th_r
    t0 = _t("threshold scan", t0)
    key_parts = []
    for c in range(NCORES):
        pr, qr = np.nonzero(masks[c])
        # composite sort key: query (11 bits) then global passage (18 bits)
        key_parts.append((qr << 18) | (pr + c * NSH))
    skey = np.concatenate(key_parts)
    skey.sort()
    rows = (skey >> 18).astype(np.int32)
    cols = (skey & ((1 << 18) - 1)).astype(np.int32)
    row_starts = np.searchsorted(rows, np.arange(Q + 1))
    t0 = _t("survivors", t0)
    # exact fp32 rescore of every survivor: blocked GEMM over query
    # stripes (gather survivor passages once, multiply by all QB queries,
    # select the matching column - BLAS-fast despite the QB-x overcompute)
    exact = np.empty(len(cols), dtype=np.float32)
    QB = 16
    for r0 in range(0, Q, QB):
        s0, e0 = row_starts[r0], row_starts[r0 + QB]
        if e0 == s0:
            continue
        sb = p[cols[s0:e0]] @ q[r0:r0 + QB].T          # [ns, QB]
        exact[s0:e0] = sb[np.arange(e0 - s0), rows[s0:e0] - r0]
    t0 = _t("rescore", t0)
    # ties -> lowest passage index, matching jax.lax.top_k
    order = np.lexsort((cols, -exact, rows))
    cols = cols[order]
    exact = exact[order]
    pick = (row_starts[:-1, None] + np.arange(k)[None, :]).ravel()
    inds = cols[pick].reshape(Q, k).astype(np.int32)
    vals = exact[pick].reshape(Q, k)
    t0 = _t("final sort", t0)
    return inds, vals
